# revision 19
# baseline (speedup 1.0000x reference)
"""Trainium2 Bass kernel for nn_Attn: softmax(out_state @ history.T, axis=-1).

Full shapes: out_state [8192, 1024] f32, history [8192, 1024] f32,
output [8192, 8192] f32.  Sharded by out_state rows across 8 cores; history
replicated.

Strategy per core (rows = 1024 out_state rows):
  - Host pre-splits both operands into fp16 hi/lo pairs (x = hi + lo exactly
    to ~2^-22 rel) and pre-transposes them to [hidden, rows] so the device
    needs no transposes: the contraction dim lands on SBUF partitions
    directly.
  - scores = A@B^T computed as 3 fp16 matmul passes accumulated in PSUM f32:
    hi*hi + lo*hi + hi*lo  (lo*lo term ~2^-22 rel, dropped).
  - Online softmax with per-512-column-chunk max: each PSUM chunk [128,512]
    is reduced (max) on DVE, then ScalarE writes exp(x - chunk_max) to an
    SBUF fp16 score buffer while accumulating the chunk sum.  At stripe end
    the chunk maxes/sums are combined into global row max/sum, and a final
    DVE pass rescales each chunk by exp(m_c - m_fin)/sum into f32 output.
"""

import numpy as np

import concourse.bass as bass
import concourse.tile as tile
from concourse import bacc, mybir
from concourse.bass_utils import run_bass_kernel_spmd

P = 128
N_CORES = 8

FP16 = mybir.dt.float16
FP32 = mybir.dt.float32
AF = mybir.ActivationFunctionType
ALU = mybir.AluOpType
AX = mybir.AxisListType


def build_attn_module(
    rows=1024,      # out_state rows per core
    seq=8192,       # history rows (output columns)
    hid=1024,       # hidden (contraction) dim
    chunk=512,      # output column chunk (<= one PSUM bank of f32)
    n_groups=2,     # stripe groups; history is streamed once per group
    psum_bufs=8,
    escore_bufs=None,
    repeat=1,       # python-unrolled repeats of the whole computation
    loop_repeat=1,  # HW For_i loop repeats (for timing harnesses)
    fixed_weights=False,  # timing-only: reuse one lhsT tile for every matmul
    mm_only=False,        # timing-only: skip softmax + output (keep B DMA)
    no_b_dma=False,       # timing-only: load B once, reuse for every chunk
    no_evac=False,        # timing-only: pure MM stream, no PSUM readers
    in_dt=16,             # 16 -> fp16 operands, else bf16
    dedupe_ldw=False,     # remove redundant consecutive identical LDWEIGHTS
    strip_incs=False,     # drop per-MM sem incs except on group-final MMs
):
    IDT = FP16 if in_dt == 16 else mybir.dt.bfloat16
    ksub = hid // P
    stripes = rows // P
    nchunk = seq // chunk
    assert stripes % n_groups == 0
    spg = stripes // n_groups
    if escore_bufs is None:
        escore_bufs = min(stripes, spg + 2)

    nc = bacc.Bacc("TRN2", target_bir_lowering=False, debug=False, num_devices=1)

    at_hi = nc.dram_tensor("at_hi", [hid, rows], IDT, kind="ExternalInput").ap()
    at_lo = nc.dram_tensor("at_lo", [hid, rows], IDT, kind="ExternalInput").ap()
    bt_hi = nc.dram_tensor("bt_hi", [hid, seq], IDT, kind="ExternalInput").ap()
    bt_lo = nc.dram_tensor("bt_lo", [hid, seq], IDT, kind="ExternalInput").ap()
    out = nc.dram_tensor("out", [rows, seq], FP32, kind="ExternalOutput").ap()

    # [hid, n] -> [P, ksub, n] so the contraction dim is on partitions
    at_hi_r = at_hi.rearrange("(ko p) r -> p ko r", p=P)
    at_lo_r = at_lo.rearrange("(ko p) r -> p ko r", p=P)
    bt_hi_r = bt_hi.rearrange("(ko p) j -> p ko j", p=P)
    bt_lo_r = bt_lo.rearrange("(ko p) j -> p ko j", p=P)

    with tile.TileContext(nc) as tc:
        with (
            tc.tile_pool(name="a_pool", bufs=1) as a_pool,
            tc.tile_pool(name="b_pool", bufs=2) as b_pool,
            tc.tile_pool(name="psum", bufs=psum_bufs, space="PSUM") as psum_pool,
            tc.tile_pool(name="escore", bufs=escore_bufs) as escore_pool,
            tc.tile_pool(name="stats", bufs=2 * stripes) as stats_pool,
            tc.tile_pool(name="fin", bufs=8) as fin_pool,
            tc.tile_pool(name="outp", bufs=4) as out_pool,
        ):
            def body():
                a_hi_t = a_pool.tile([P, ksub, rows], IDT, tag="a_hi")
                nc.sync.dma_start(a_hi_t[:], at_hi_r[:])
                a_lo_t = a_pool.tile([P, ksub, rows], IDT, tag="a_lo")
                nc.sync.dma_start(a_lo_t[:], at_lo_r[:])

                for g in range(n_groups):
                    g_stripes = range(g * spg, (g + 1) * spg)
                    negm = {}   # [P, nchunk] f32, -chunk_max per chunk
                    ssum = {}   # [P, nchunk] f32, sum(exp(x - chunk_max))
                    escore = {}  # [P, seq] fp16, exp(x - chunk_max)
                    for s in g_stripes:
                        negm[s] = stats_pool.tile([P, nchunk], FP32, tag="negm", name=f"negm_{s}")
                        ssum[s] = stats_pool.tile([P, nchunk], FP32, tag="ssum", name=f"ssum_{s}")
                        escore[s] = escore_pool.tile([P, seq], FP16, tag="escore", name=f"escore_{s}")

                    b_fixed = {}
                    if no_b_dma:
                        b_fixed["hi"] = b_pool.tile(
                            [P, ksub, chunk], IDT, tag="b_hi", name="b_hi_fix"
                        )
                        nc.sync.dma_start(b_fixed["hi"][:], bt_hi_r[:, :, 0:chunk])
                        b_fixed["lo"] = b_pool.tile(
                            [P, ksub, chunk], IDT, tag="b_lo", name="b_lo_fix"
                        )
                        nc.sync.dma_start(b_fixed["lo"][:], bt_lo_r[:, :, 0:chunk])

                    for c in range(nchunk):
                        if no_b_dma:
                            b_hi_t, b_lo_t = b_fixed["hi"], b_fixed["lo"]
                        else:
                            b_hi_t = b_pool.tile([P, ksub, chunk], IDT, tag="b_hi")
                            nc.sync.dma_start(
                                b_hi_t[:], bt_hi_r[:, :, c * chunk:(c + 1) * chunk]
                            )
                            b_lo_t = b_pool.tile([P, ksub, chunk], IDT, tag="b_lo")
                            nc.sync.dma_start(
                                b_lo_t[:], bt_lo_r[:, :, c * chunk:(c + 1) * chunk]
                            )
                        for s in g_stripes:
                            ps = psum_pool.tile([P, chunk], FP32, tag="ps")
                            n_mm = 3 * ksub
                            i_mm = 0
                            for a_t, b_t in (
                                (a_hi_t, b_hi_t),
                                (a_lo_t, b_hi_t),
                                (a_hi_t, b_lo_t),
                            ):
                                for k in range(ksub):
                                    if fixed_weights:
                                        lhsT = a_hi_t[:, 0, 0:P]
                                    else:
                                        lhsT = a_t[:, k, s * P:(s + 1) * P]
                                    nc.tensor.matmul(
                                        ps[:],
                                        lhsT=lhsT,
                                        rhs=b_t[:, k, :],
                                        start=(i_mm == 0),
                                        stop=(i_mm == n_mm - 1),
                                    )
                                    i_mm += 1
                            if mm_only:
                                if not no_evac:
                                    # evacuate PSUM minimally so banks recycle
                                    nc.vector.tensor_reduce(
                                        negm[s][:, c:c + 1], ps[:],
                                        axis=AX.X, op=ALU.max, negate=True,
                                    )
                                continue
                            # -max of chunk
                            nc.vector.tensor_reduce(
                                negm[s][:, c:c + 1], ps[:],
                                axis=AX.X, op=ALU.max, negate=True,
                            )
                            # exp(x - max) -> fp16 scores; chunk sum on the side
                            nc.scalar.activation(
                                escore[s][:, c * chunk:(c + 1) * chunk],
                                ps[:],
                                AF.Exp,
                                bias=negm[s][:, c:c + 1],
                                accum_out=ssum[s][:, c:c + 1],
                            )

                    for s in (() if mm_only else g_stripes):
                        # -m_fin = min_c(-m_c)
                        negm_fin = fin_pool.tile([P, 1], FP32, tag="negm_fin")
                        nc.vector.tensor_reduce(
                            negm_fin[:], negm[s][:], axis=AX.X, op=ALU.min,
                        )
                        # etab_c = exp(m_c - m_fin) = exp(-negm_c + negm_fin)
                        etab = fin_pool.tile([P, nchunk], FP32, tag="etab")
                        nc.vector.tensor_scalar(
                            etab[:], negm[s][:], -1.0, negm_fin[:],
                            op0=ALU.mult, op1=ALU.add,
                        )
                        nc.scalar.activation(etab[:], etab[:], AF.Exp)
                        # s_fin = sum_c ssum_c * etab_c
                        prod = fin_pool.tile([P, nchunk], FP32, tag="prod")
                        nc.vector.tensor_tensor(
                            prod[:], ssum[s][:], etab[:], op=ALU.mult
                        )
                        sfin = fin_pool.tile([P, 1], FP32, tag="sfin")
                        nc.vector.tensor_reduce(
                            sfin[:], prod[:], axis=AX.X, op=ALU.add,
                        )
                        rec = fin_pool.tile([P, 1], FP32, tag="rec")
                        nc.vector.reciprocal(rec[:], sfin[:])
                        # factor_c = etab_c / s_fin
                        fact = fin_pool.tile([P, nchunk], FP32, tag="fact")
                        nc.vector.tensor_scalar(
                            fact[:], etab[:], rec[:], None, op0=ALU.mult,
                        )
                        for c in range(nchunk):
                            ot = out_pool.tile([P, chunk], FP32, tag="ot")
                            nc.vector.tensor_scalar(
                                ot[:],
                                escore[s][:, c * chunk:(c + 1) * chunk],
                                fact[:, c:c + 1], None, op0=ALU.mult,
                            )
                            nc.sync.dma_start(
                                out[s * P:(s + 1) * P, c * chunk:(c + 1) * chunk],
                                ot[:],
                            )

            if loop_repeat > 1:
                with tc.For_i(0, loop_repeat, 1):
                    body()
            else:
                for _ in range(repeat):
                    body()

    if dedupe_ldw:
        _dedupe_ldweights(nc)
    if strip_incs:
        _strip_mm_sem_incs(nc)
    nc.compile()
    return nc


def _strip_mm_sem_incs(nc):
    """Drop the per-matmul semaphore increment on non-group-final matmuls.

    Tile puts `S[PE] += 1` on every matmul; each inc is a serialized EVT_SEM
    register write (~26ns) on the PE queue.  Matmuls complete in program
    order, so consumers only ever need the group-final matmul's increment.
    Keeping increments only on `stop_tensor_calc=True` matmuls (and any
    non-matmul PE updates) preserves ordering semantics provided every wait
    value is remapped onto the surviving increment sequence, rounding up to
    the next kept increment (which can only make a waiter later, i.e. safe).
    Sems whose updates span multiple blocks or use non-inc modes are left
    untouched.
    """
    for fn in nc.m.functions:
        # sem id -> block name -> list of (inst, kept)
        upd_by_sem = {}
        bad_sems = set()
        blocks = list(fn.blocks)
        for blk in blocks:
            for inst in blk.instructions:
                si = inst.sync_info
                if not si or not si.on_update:
                    continue
                for u in si.on_update:
                    if u.sync_type != "semaphore":
                        continue
                    if u.update_mode != "sem-inc" or u.update_value != 1:
                        bad_sems.add(u.id)
                        continue
                    is_mm = type(inst).__name__ == "InstMatmult"
                    kept = (not is_mm) or bool(inst.stop_tensor_calc)
                    upd_by_sem.setdefault(u.id, {}).setdefault(
                        blk.name, []
                    ).append((inst, kept))
        # collect waits per sem across blocks
        wait_sites = {}
        for blk in blocks:
            for inst in blk.instructions:
                si = inst.sync_info
                if not si or not si.on_wait:
                    continue
                for w in si.on_wait:
                    if w.sync_type == "semaphore":
                        wait_sites.setdefault(w.id, []).append((blk.name, inst, w))

        for sem_id, per_block in upd_by_sem.items():
            if sem_id in bad_sems or len(per_block) != 1:
                continue
            (blk_name, updates), = per_block.items()
            n = len(updates)
            n_stripped = sum(1 for _, kept in updates if not kept)
            if n_stripped == 0:
                continue
            # waits on this sem must all be ge-mode and either in the same
            # block or target the final value
            sites = wait_sites.get(sem_id, [])
            ok = all(
                w.wait_mode == "sem-ge-imm"
                and (bn == blk_name or w.wait_value >= n)
                for bn, _, w in sites
            )
            if not ok:
                continue
            # ensure the final update is kept
            updates[-1] = (updates[-1][0], True)
            # prefix counts of kept updates
            kept_prefix = []
            kc = 0
            for _, kept in updates:
                kc += kept
                kept_prefix.append(kc)
            total_new = kc

            def remap(v):
                if v <= 0:
                    return v
                j = min(v, n) - 1
                # find first kept update at index >= j
                while j < n and kept_prefix[j] == (kept_prefix[j - 1] if j else 0):
                    j += 1
                if j >= n:
                    return total_new
                return kept_prefix[j]

            for bn, inst, w in sites:
                w.wait_value = remap(w.wait_value)
            for inst, kept in updates:
                if kept:
                    continue
                si = inst.sync_info
                si.on_update = [
                    u for u in si.on_update
                    if not (u.sync_type == "semaphore" and u.id == sem_id)
                ]


def _dedupe_ldweights(nc):
    """Delete InstLdweights that reload the exact weights already resident.

    Tile lowering emits one LDW per matmul even when consecutive matmuls use
    the identical stationary tile.  A redundant LDW with no semaphore
    waits/updates is a pure no-op for program semantics; removing it frees
    ~53ns of PE issue time per matmul.
    """
    n_removed = 0
    for fn in nc.m.functions:
        for blk in fn.blocks:
            insts = list(blk.instructions)
            keep = []
            last_ldw_key = None
            for inst in insts:
                tn = type(inst).__name__
                if getattr(inst, "engine", None) == mybir.EngineType.PE:
                    if tn == "InstLdweights":
                        key = inst.ins[0].concise()
                        si = inst.sync_info
                        clean = not si or (not si.on_wait and not si.on_update)
                        if clean and key == last_ldw_key:
                            n_removed += 1
                            continue  # drop it
                        last_ldw_key = key
                    elif tn != "InstMatmult":
                        # any other PE instruction invalidates the array state
                        last_ldw_key = None
                keep.append(inst)
            if len(keep) != len(insts):
                blk.instructions = keep
    return n_removed


def _split_t(m: np.ndarray):
    """f32 [r, h] -> (hi, lo) fp16, each [h, r] (transposed), x = hi + lo."""
    hi = m.astype(np.float16)
    lo = (m - hi.astype(np.float32)).astype(np.float16)
    return np.ascontiguousarray(hi.T), np.ascontiguousarray(lo.T)


_module_cache = {}


def _get_module(**kw):
    key = tuple(sorted(kw.items()))
    if key not in _module_cache:
        _module_cache[key] = build_attn_module(**kw)
    return _module_cache[key]


def kernel(out_state: np.ndarray, history: np.ndarray) -> np.ndarray:
    out_state = np.asarray(out_state, dtype=np.float32)
    history = np.asarray(history, dtype=np.float32)
    state_len, hid = out_state.shape
    seq = history.shape[0]
    rows = state_len // N_CORES

    bt_hi, bt_lo = _split_t(history)
    in_maps = []
    for c in range(N_CORES):
        at_hi, at_lo = _split_t(out_state[c * rows:(c + 1) * rows])
        in_maps.append(
            {"at_hi": at_hi, "at_lo": at_lo, "bt_hi": bt_hi, "bt_lo": bt_lo}
        )

    nc = _get_module(rows=rows, seq=seq, hid=hid)
    res = run_bass_kernel_spmd(nc, in_maps, list(range(N_CORES)))
    return np.concatenate(
        [res.results[c]["out"] for c in range(N_CORES)], axis=0
    )


# revision 24
# speedup vs baseline: 1.5289x; 1.5289x over previous
"""Trainium2 Bass kernel for nn_Attn: softmax(out_state @ history.T, axis=-1).

Full shapes: out_state [8192, 1024] f32, history [8192, 1024] f32,
output [8192, 8192] f32.  Sharded by out_state rows across 8 cores; history
replicated.

Strategy per core (rows = 1024 out_state rows):
  - Host pre-splits both operands into fp16 hi/lo pairs (x = hi + lo exactly
    to ~2^-22 rel) and pre-transposes them to [hidden, rows] so the device
    needs no transposes: the contraction dim lands on SBUF partitions
    directly.
  - scores = A@B^T computed as 3 fp16 matmul passes accumulated in PSUM f32:
    hi*hi + lo*hi + hi*lo  (lo*lo term ~2^-22 rel, dropped).
  - Online softmax with per-512-column-chunk max: each PSUM chunk [128,512]
    is reduced (max) on DVE, then ScalarE writes exp(x - chunk_max) to an
    SBUF fp16 score buffer while accumulating the chunk sum.  At stripe end
    the chunk maxes/sums are combined into global row max/sum, and a final
    DVE pass rescales each chunk by exp(m_c - m_fin)/sum into f32 output.
"""

import numpy as np

import concourse.bass as bass
import concourse.tile as tile
from concourse import bacc, mybir
from concourse.bass_utils import run_bass_kernel_spmd

P = 128
N_CORES = 8

FP16 = mybir.dt.float16
FP32 = mybir.dt.float32
AF = mybir.ActivationFunctionType
ALU = mybir.AluOpType
AX = mybir.AxisListType


def build_attn_module(
    rows=1024,      # out_state rows per core
    seq=8192,       # history rows (output columns)
    hid=1024,       # hidden (contraction) dim
    chunk=512,      # output column chunk (<= one PSUM bank of f32)
    n_groups=2,     # stripe groups; history is streamed once per group
    psum_bufs=8,
    escore_bufs=None,
    repeat=1,       # python-unrolled repeats of the whole computation
    loop_repeat=1,  # HW For_i loop repeats (for timing harnesses)
    fixed_weights=False,  # timing-only: reuse one lhsT tile for every matmul
    mm_only=False,        # timing-only: skip softmax + output (keep B DMA)
    no_b_dma=False,       # timing-only: load B once, reuse for every chunk
    no_evac=False,        # timing-only: pure MM stream, no PSUM readers
    in_dt=16,             # 16 -> fp16 operands, else bf16
    dedupe_ldw=False,     # remove redundant consecutive identical LDWEIGHTS
    strip_incs=False,     # drop per-MM sem incs except on group-final MMs
    cw=1,                 # chunks computed per weight load (weight reuse)
):
    IDT = FP16 if in_dt == 16 else mybir.dt.bfloat16
    ksub = hid // P
    stripes = rows // P
    nchunk = seq // chunk
    assert stripes % n_groups == 0
    spg = stripes // n_groups
    if escore_bufs is None:
        escore_bufs = min(stripes, spg + 2)

    nc = bacc.Bacc("TRN2", target_bir_lowering=False, debug=False, num_devices=1)

    at_hi = nc.dram_tensor("at_hi", [hid, rows], IDT, kind="ExternalInput").ap()
    at_lo = nc.dram_tensor("at_lo", [hid, rows], IDT, kind="ExternalInput").ap()
    bt_hi = nc.dram_tensor("bt_hi", [hid, seq], IDT, kind="ExternalInput").ap()
    bt_lo = nc.dram_tensor("bt_lo", [hid, seq], IDT, kind="ExternalInput").ap()
    out = nc.dram_tensor("out", [rows, seq], FP32, kind="ExternalOutput").ap()

    # [hid, n] -> [P, ksub, n] so the contraction dim is on partitions
    at_hi_r = at_hi.rearrange("(ko p) r -> p ko r", p=P)
    at_lo_r = at_lo.rearrange("(ko p) r -> p ko r", p=P)
    bt_hi_r = bt_hi.rearrange("(ko p) j -> p ko j", p=P)
    bt_lo_r = bt_lo.rearrange("(ko p) j -> p ko j", p=P)

    with tile.TileContext(nc) as tc:
        with (
            tc.tile_pool(name="a_pool", bufs=1) as a_pool,
            tc.tile_pool(name="b_pool", bufs=2 * cw) as b_pool,
            tc.tile_pool(name="psum", bufs=psum_bufs, space="PSUM") as psum_pool,
            tc.tile_pool(name="escore", bufs=escore_bufs) as escore_pool,
            tc.tile_pool(name="stats", bufs=2 * stripes) as stats_pool,
            tc.tile_pool(name="fin", bufs=8) as fin_pool,
            tc.tile_pool(name="outp", bufs=4) as out_pool,
        ):
            def body():
                a_hi_t = a_pool.tile([P, ksub, rows], IDT, tag="a_hi")
                nc.sync.dma_start(a_hi_t[:], at_hi_r[:])
                a_lo_t = a_pool.tile([P, ksub, rows], IDT, tag="a_lo")
                nc.sync.dma_start(a_lo_t[:], at_lo_r[:])

                for g in range(n_groups):
                    g_stripes = range(g * spg, (g + 1) * spg)
                    negm = {}   # [P, nchunk] f32, -chunk_max per chunk
                    ssum = {}   # [P, nchunk] f32, sum(exp(x - chunk_max))
                    escore = {}  # [P, seq] fp16, exp(x - chunk_max)
                    for s in g_stripes:
                        negm[s] = stats_pool.tile([P, nchunk], FP32, tag="negm", name=f"negm_{s}")
                        ssum[s] = stats_pool.tile([P, nchunk], FP32, tag="ssum", name=f"ssum_{s}")
                        escore[s] = escore_pool.tile([P, seq], FP16, tag="escore", name=f"escore_{s}")

                    b_fixed = {}
                    if no_b_dma:
                        b_fixed["hi"] = b_pool.tile(
                            [P, ksub, chunk], IDT, tag="b_hi", name="b_hi_fix"
                        )
                        nc.sync.dma_start(b_fixed["hi"][:], bt_hi_r[:, :, 0:chunk])
                        b_fixed["lo"] = b_pool.tile(
                            [P, ksub, chunk], IDT, tag="b_lo", name="b_lo_fix"
                        )
                        nc.sync.dma_start(b_fixed["lo"][:], bt_lo_r[:, :, 0:chunk])

                    for cp in range(nchunk // cw):
                        cs = [cp * cw + i for i in range(cw)]
                        if no_b_dma:
                            b_his = [b_fixed["hi"]] * cw
                            b_los = [b_fixed["lo"]] * cw
                        else:
                            b_his, b_los = [], []
                            for c in cs:
                                bh = b_pool.tile(
                                    [P, ksub, chunk], IDT, tag="b_hi",
                                    name=f"b_hi_{c}",
                                )
                                nc.sync.dma_start(
                                    bh[:], bt_hi_r[:, :, c * chunk:(c + 1) * chunk]
                                )
                                bl = b_pool.tile(
                                    [P, ksub, chunk], IDT, tag="b_lo",
                                    name=f"b_lo_{c}",
                                )
                                nc.sync.dma_start(
                                    bl[:], bt_lo_r[:, :, c * chunk:(c + 1) * chunk]
                                )
                                b_his.append(bh)
                                b_los.append(bl)
                        for s in g_stripes:
                            pss = [
                                psum_pool.tile(
                                    [P, chunk], FP32, tag="ps", name=f"ps_{c}"
                                )
                                for c in cs
                            ]
                            n_mm = 3 * ksub
                            i_mm = 0
                            for a_t, b_ts in (
                                (a_hi_t, b_his),
                                (a_lo_t, b_his),
                                (a_hi_t, b_los),
                            ):
                                for k in range(ksub):
                                    if fixed_weights:
                                        lhsT = a_hi_t[:, 0, 0:P]
                                    else:
                                        lhsT = a_t[:, k, s * P:(s + 1) * P]
                                    for i in range(cw):
                                        nc.tensor.matmul(
                                            pss[i][:],
                                            lhsT=lhsT,
                                            rhs=b_ts[i][:, k, :],
                                            start=(i_mm == 0),
                                            stop=(i_mm == n_mm - 1),
                                        )
                                    i_mm += 1
                            for i, c in enumerate(cs):
                                ps = pss[i]
                                if mm_only:
                                    if not no_evac:
                                        nc.vector.tensor_reduce(
                                            negm[s][:, c:c + 1], ps[:],
                                            axis=AX.X, op=ALU.max, negate=True,
                                        )
                                    continue
                                # -max of chunk
                                nc.vector.tensor_reduce(
                                    negm[s][:, c:c + 1], ps[:],
                                    axis=AX.X, op=ALU.max, negate=True,
                                )
                                # exp(x - max) -> fp16 scores; chunk sum aside
                                nc.scalar.activation(
                                    escore[s][:, c * chunk:(c + 1) * chunk],
                                    ps[:],
                                    AF.Exp,
                                    bias=negm[s][:, c:c + 1],
                                    accum_out=ssum[s][:, c:c + 1],
                                )

                    for s in (() if mm_only else g_stripes):
                        # -m_fin = min_c(-m_c)
                        negm_fin = fin_pool.tile([P, 1], FP32, tag="negm_fin")
                        nc.vector.tensor_reduce(
                            negm_fin[:], negm[s][:], axis=AX.X, op=ALU.min,
                        )
                        # etab_c = exp(m_c - m_fin) = exp(-negm_c + negm_fin)
                        etab = fin_pool.tile([P, nchunk], FP32, tag="etab")
                        nc.vector.tensor_scalar(
                            etab[:], negm[s][:], -1.0, negm_fin[:],
                            op0=ALU.mult, op1=ALU.add,
                        )
                        nc.scalar.activation(etab[:], etab[:], AF.Exp)
                        # s_fin = sum_c ssum_c * etab_c
                        prod = fin_pool.tile([P, nchunk], FP32, tag="prod")
                        nc.vector.tensor_tensor(
                            prod[:], ssum[s][:], etab[:], op=ALU.mult
                        )
                        sfin = fin_pool.tile([P, 1], FP32, tag="sfin")
                        nc.vector.tensor_reduce(
                            sfin[:], prod[:], axis=AX.X, op=ALU.add,
                        )
                        rec = fin_pool.tile([P, 1], FP32, tag="rec")
                        nc.vector.reciprocal(rec[:], sfin[:])
                        # factor_c = etab_c / s_fin
                        fact = fin_pool.tile([P, nchunk], FP32, tag="fact")
                        nc.vector.tensor_scalar(
                            fact[:], etab[:], rec[:], None, op0=ALU.mult,
                        )
                        for c in range(nchunk):
                            ot = out_pool.tile([P, chunk], FP32, tag="ot")
                            nc.vector.tensor_scalar(
                                ot[:],
                                escore[s][:, c * chunk:(c + 1) * chunk],
                                fact[:, c:c + 1], None, op0=ALU.mult,
                            )
                            nc.sync.dma_start(
                                out[s * P:(s + 1) * P, c * chunk:(c + 1) * chunk],
                                ot[:],
                            )

            if loop_repeat > 1:
                with tc.For_i(0, loop_repeat, 1):
                    body()
            else:
                for _ in range(repeat):
                    body()

    if dedupe_ldw:
        _dedupe_ldweights(nc)
    if strip_incs:
        _strip_mm_sem_incs(nc)
    nc.compile()
    return nc


def _strip_mm_sem_incs(nc):
    """Drop the per-matmul semaphore increment on non-group-final matmuls.

    Tile puts `S[PE] += 1` on every matmul; each inc is a serialized EVT_SEM
    register write (~26ns) on the PE queue.  Matmuls complete in program
    order, so consumers only ever need the group-final matmul's increment.
    Keeping increments only on `stop_tensor_calc=True` matmuls (and any
    non-matmul PE updates) preserves ordering semantics provided every wait
    value is remapped onto the surviving increment sequence, rounding up to
    the next kept increment (which can only make a waiter later, i.e. safe).
    Sems whose updates span multiple blocks or use non-inc modes are left
    untouched.
    """
    for fn in nc.m.functions:
        # sem id -> block name -> list of (inst, kept)
        upd_by_sem = {}
        bad_sems = set()
        blocks = list(fn.blocks)
        for blk in blocks:
            for inst in blk.instructions:
                si = inst.sync_info
                if not si or not si.on_update:
                    continue
                for u in si.on_update:
                    if u.sync_type != "semaphore":
                        continue
                    if u.update_mode != "sem-inc" or u.update_value != 1:
                        bad_sems.add(u.id)
                        continue
                    is_mm = type(inst).__name__ == "InstMatmult"
                    kept = (not is_mm) or bool(inst.stop_tensor_calc)
                    upd_by_sem.setdefault(u.id, {}).setdefault(
                        blk.name, []
                    ).append((inst, kept))
        # collect waits per sem across blocks
        wait_sites = {}
        for blk in blocks:
            for inst in blk.instructions:
                si = inst.sync_info
                if not si or not si.on_wait:
                    continue
                for w in si.on_wait:
                    if w.sync_type == "semaphore":
                        wait_sites.setdefault(w.id, []).append((blk.name, inst, w))

        for sem_id, per_block in upd_by_sem.items():
            if sem_id in bad_sems or len(per_block) != 1:
                continue
            (blk_name, updates), = per_block.items()
            n = len(updates)
            n_stripped = sum(1 for _, kept in updates if not kept)
            if n_stripped == 0:
                continue
            # waits on this sem must all be ge-mode and either in the same
            # block or target the final value
            sites = wait_sites.get(sem_id, [])
            ok = all(
                w.wait_mode == "sem-ge-imm"
                and (bn == blk_name or w.wait_value >= n)
                for bn, _, w in sites
            )
            if not ok:
                continue
            # ensure the final update is kept
            updates[-1] = (updates[-1][0], True)
            # prefix counts of kept updates
            kept_prefix = []
            kc = 0
            for _, kept in updates:
                kc += kept
                kept_prefix.append(kc)
            total_new = kc

            def remap(v):
                if v <= 0:
                    return v
                j = min(v, n) - 1
                # find first kept update at index >= j
                while j < n and kept_prefix[j] == (kept_prefix[j - 1] if j else 0):
                    j += 1
                if j >= n:
                    return total_new
                return kept_prefix[j]

            for bn, inst, w in sites:
                w.wait_value = remap(w.wait_value)
            for inst, kept in updates:
                if kept:
                    continue
                si = inst.sync_info
                si.on_update = [
                    u for u in si.on_update
                    if not (u.sync_type == "semaphore" and u.id == sem_id)
                ]


def _dedupe_ldweights(nc):
    """Delete InstLdweights that reload the exact weights already resident.

    Tile lowering emits one LDW per matmul even when consecutive matmuls use
    the identical stationary tile.  A redundant LDW with no semaphore
    waits/updates is a pure no-op for program semantics; removing it frees
    ~53ns of PE issue time per matmul.
    """
    n_removed = 0
    for fn in nc.m.functions:
        for blk in fn.blocks:
            insts = list(blk.instructions)
            # sanity: every matmul must consume the weights loaded by the
            # nearest preceding LDW, else pairing assumptions are broken
            last_key = None
            consistent = True
            for inst in insts:
                if getattr(inst, "engine", None) != mybir.EngineType.PE:
                    continue
                tn = type(inst).__name__
                if tn == "InstLdweights":
                    last_key = inst.ins[0].concise()
                elif tn == "InstMatmult":
                    if len(inst.ins) > 1 and last_key is not None:
                        if inst.ins[1].concise() != last_key:
                            consistent = False
                            break
                else:
                    last_key = None
            if not consistent:
                continue
            keep = []
            last_ldw_key = None
            for inst in insts:
                tn = type(inst).__name__
                if getattr(inst, "engine", None) == mybir.EngineType.PE:
                    if tn == "InstLdweights":
                        key = inst.ins[0].concise()
                        si = inst.sync_info
                        clean = not si or (not si.on_wait and not si.on_update)
                        if clean and key == last_ldw_key:
                            n_removed += 1
                            continue  # drop it
                        last_ldw_key = key
                    elif tn != "InstMatmult":
                        # any other PE instruction invalidates the array state
                        last_ldw_key = None
                keep.append(inst)
            if len(keep) != len(insts):
                blk.instructions = keep
    return n_removed


def _split_t(m: np.ndarray):
    """f32 [r, h] -> (hi, lo) fp16, each [h, r] (transposed), x = hi + lo."""
    hi = m.astype(np.float16)
    lo = (m - hi.astype(np.float32)).astype(np.float16)
    return np.ascontiguousarray(hi.T), np.ascontiguousarray(lo.T)


_module_cache = {}


def _get_module(**kw):
    key = tuple(sorted(kw.items()))
    if key not in _module_cache:
        _module_cache[key] = build_attn_module(**kw)
    return _module_cache[key]


def kernel(out_state: np.ndarray, history: np.ndarray) -> np.ndarray:
    out_state = np.asarray(out_state, dtype=np.float32)
    history = np.asarray(history, dtype=np.float32)
    state_len, hid = out_state.shape
    seq = history.shape[0]
    rows = state_len // N_CORES

    bt_hi, bt_lo = _split_t(history)
    in_maps = []
    for c in range(N_CORES):
        at_hi, at_lo = _split_t(out_state[c * rows:(c + 1) * rows])
        in_maps.append(
            {"at_hi": at_hi, "at_lo": at_lo, "bt_hi": bt_hi, "bt_lo": bt_lo}
        )

    nc = _get_module(rows=rows, seq=seq, hid=hid)
    res = run_bass_kernel_spmd(nc, in_maps, list(range(N_CORES)))
    return np.concatenate(
        [res.results[c]["out"] for c in range(N_CORES)], axis=0
    )
